# revision 8
# baseline (speedup 1.0000x reference)
"""Entity-aware BERT attention (LUKE-style) Trainium2 Bass kernel.

Sharding: 8 cores = 4 batches x 2 query-halves. Each core computes the full
K/V for its batch (duplicated across the pair of cores sharing a batch, which
avoids any cross-core collective), then attention + output projection +
LayerNorm for its own 512 word rows and 64 entity rows.

All matmuls run in float32r (full PE rate for free-dim >= 256). Activations
are kept transposed (hidden on partitions) so projections and attention feed
the tensor engine directly; scores are computed transposed (S^T[kv, q]) so
the softmax exp output is consumed as the matmul moving operand with no
transpose. The softmax denominator comes from an appended ones-column in V,
and the per-query division uses a K=1 matmul to broadcast 1/denom across
partitions. Free-dim biases are applied via partition-broadcast DMA tiles.
"""
import os
import sys

sys.path.insert(0, "/opt/trn_rl_repo")

import numpy as np  # noqa: E402

import concourse.bass as bass  # noqa: E402,F401
import concourse.tile as tile  # noqa: E402
from concourse import bacc, mybir  # noqa: E402
from concourse.bass_utils import run_bass_kernel_spmd  # noqa: E402
from concourse.masks import make_identity  # noqa: E402
from contextlib import ExitStack  # noqa: E402

F32 = mybir.dt.float32
F32R = mybir.dt.float32r
AF = mybir.ActivationFunctionType
OP = mybir.AluOpType

P = 128
H = 1024
NH = 16
DH = 64
TW = 1024        # word tokens
TE = 128         # entity tokens
T = TW + TE      # 1152 keys
QWR = 512        # word query rows per core
QER = 64         # entity query rows per core
NQ = QWR + QER   # 576 queries per core
HC = H // P      # 8 hidden chunks
KC = T // P      # 9 kv chunks
QH = NQ // 2     # 288, matmul free-dim half (>=256 keeps fp32r at full rate)
NQT = 256        # weight streaming quarter width
EPS = 1e-12

_STATE = {}
LAST_RESULTS = None


def _build():
    nc = bacc.Bacc("TRN2", target_bir_lowering=False, debug=False, num_devices=8)

    d = {}
    def din(name, shape):
        d[name] = nc.dram_tensor(name, shape, F32, kind="ExternalInput").ap()
    def dout(name, shape):
        d[name] = nc.dram_tensor(name, shape, F32, kind="ExternalOutput").ap()

    din("word", [TW, H]); din("ent", [TE, H]); din("qpos", [TE, H])
    din("word_q", [QWR, H]); din("ent_q", [QER, H]); din("qpos_q", [QER, H])
    din("mask", [T])
    for w in ["Wq", "Wk", "Wv", "Weq", "Wek", "Wev", "Wo", "Weo"]:
        din(w, [H, H])
    for b in ["bq", "bk", "bv", "beq", "bek", "bev", "bo", "beo"]:
        din(b, [H])
    for g in ["ln_g", "ln_b", "eln_g", "eln_b"]:
        din(g, [H])
    dout("word_out", [QWR, H])
    dout("ent_out", [QER, H])

    with tile.TileContext(nc) as tc:
        _emit(nc, tc, d)
    nc.compile()
    return nc


def _emit(nc, tc, d):
    with ExitStack() as ctx0:
        const = ctx0.enter_context(tc.tile_pool(name="const", bufs=1))
        ctxp = ctx0.enter_context(tc.tile_pool(name="ctxp", bufs=1))

        # ---- constants ----
        ident = const.tile([P, P], F32)
        make_identity(nc, ident)
        ident_r = const.tile([P, P], F32R)
        nc.gpsimd.tensor_copy(ident_r, ident)
        ones_f = const.tile([1, P], F32)
        nc.vector.memset(ones_f, 1.0)
        ones_r = const.tile([1, P], F32R)
        nc.vector.tensor_copy(ones_r, ones_f)
        eps_t = const.tile([P, 1], F32)
        nc.vector.memset(eps_t, EPS)

        maskT = const.tile([P, KC], F32)
        nc.sync.dma_start(maskT, d["mask"].rearrange("(c p) -> p c", p=P))

        # per-partition (feature-on-partition) bias views for transposed outs
        pbias = {}
        for b in ["bk", "bq", "bek", "beq"]:
            t = const.tile([P, HC], F32, name=f"pb_{b}")
            nc.sync.dma_start(t, d[b].rearrange("(m p) -> p m", p=P))
            pbias[b] = t

        def bias_bcast(pool, name, tag):
            # [P, H] partition-broadcast copy of a [H] bias row
            t = pool.tile([P, H], F32, tag=tag, name=f"bb_{name}")
            nc.sync.dma_start(t, d[name][None, :].to_broadcast((P, H)))
            return t

        ctxT = ctxp.tile([P, HC, NQ], F32R)     # attention output^T [feat, q]

        with ExitStack() as ctxKVQ:
            kvq = ctxKVQ.enter_context(tc.tile_pool(name="kvq", bufs=1))
            KT = kvq.tile([P, HC, T], F32R)          # K^T  [feat, kv]
            V = kvq.tile([P, KC, NH, DH + 1], F32R)  # V (+ones col)
            QT = kvq.tile([P, HC, NQ], F32R)         # Q^T / 8  [feat, q]
            vone_f = const.tile([P, KC, NH, 1], F32)
            nc.vector.memset(vone_f, 1.0)
            nc.vector.tensor_copy(V[:, :, :, DH:DH + 1], vone_f)

            # ============ phase 1+2: transposes & projections ============
            with ExitStack() as ctxB:
                natp = ctxB.enter_context(tc.tile_pool(name="nat", bufs=2))
                wt32 = ctxB.enter_context(tc.tile_pool(name="wt32", bufs=2))
                wtr = ctxB.enter_context(tc.tile_pool(name="wtr", bufs=2))
                bvp = ctxB.enter_context(tc.tile_pool(name="bvp", bufs=2))
                tpps = ctxB.enter_context(
                    tc.tile_pool(name="tpps", bufs=2, space="PSUM"))
                prps = ctxB.enter_context(
                    tc.tile_pool(name="prps", bufs=4, space="PSUM"))

                def transpose_block(dst_ap, src_ap, rows, use_r=False,
                                    scale_bias=None):
                    ps = tpps.tile([P, P], F32R if use_r else F32, tag="tp",
                                   name="tps")
                    idt = ident_r if use_r else ident
                    nc.tensor.transpose(ps[:, :rows], src_ap, idt[:rows, :rows])
                    if scale_bias is None:
                        nc.vector.tensor_copy(dst_ap, ps[:, :rows])
                    else:
                        b_ap, sc = scale_bias
                        nc.vector.tensor_scalar(
                            out=dst_ap, in0=ps[:, :rows], scalar1=b_ap,
                            scalar2=sc, op0=OP.add, op1=OP.mult)

                def load_w(name, q):
                    wt = wt32.tile([P, HC, NQT], F32, tag="wt", name="wt")
                    nc.sync.dma_start(
                        wt, d[name].rearrange("(k p) f -> p k f", p=P)
                        [:, :, q * NQT:(q + 1) * NQT])
                    wr = wtr.tile([P, HC, NQT], F32R, tag="wr", name="wr")
                    nc.gpsimd.tensor_copy(wr, wt)
                    return wr

                # ---- phase 2a: word^T, K^T, V (wordT freed after) ----
                with ExitStack() as ctxA:
                    wordT_pool = ctxA.enter_context(
                        tc.tile_pool(name="wordT", bufs=1))
                    wordT = wordT_pool.tile([P, HC, TW], F32R)
                    for o in range(HC):
                        nat = natp.tile([P, H], F32, tag="nat", name="nat")
                        nc.sync.dma_start(nat, d["word"][o * P:(o + 1) * P, :])
                        for c in range(HC):
                            transpose_block(wordT[:, c, o * P:(o + 1) * P],
                                            nat[:, c * P:(c + 1) * P], P)

                    # K^T = (word @ Wk)^T + bk -> KT[:, m, 0:TW]
                    for q in range(H // NQT):
                        wr = load_w("Wk", q)
                        for mloc in range(NQT // P):
                            m = q * (NQT // P) + mloc
                            for n in range(TW // 512):
                                ps = prps.tile([P, 512], F32, tag="pr",
                                               name="psk")
                                for k in range(HC):
                                    nc.tensor.matmul(
                                        ps, wr[:, k, mloc * P:(mloc + 1) * P],
                                        wordT[:, k, n * 512:(n + 1) * 512],
                                        start=(k == 0), stop=(k == HC - 1))
                                nc.vector.tensor_scalar_add(
                                    out=KT[:, m, n * 512:(n + 1) * 512],
                                    in0=ps, scalar1=pbias["bk"][:, m:m + 1])

                    # V = word @ Wv + bv (natural) -> V[:, t, :, 0:DH]
                    bvb = bias_bcast(bvp, "bv", "bv")
                    for q in range(H // NQT):
                        wr = load_w("Wv", q)
                        for t in range(TW // P):
                            ps = prps.tile([P, 512], F32, tag="pr",
                                           name="psv")[:, :NQT]
                            for k in range(HC):
                                nc.tensor.matmul(
                                    ps, wordT[:, k, t * P:(t + 1) * P],
                                    wr[:, k, :],
                                    start=(k == 0), stop=(k == HC - 1))
                            nc.vector.tensor_add(
                                V[:, t, q * 4:(q + 1) * 4, :DH],
                                ps.rearrange("p (h e) -> p h e", e=DH),
                                bvb[:, q * NQT:(q + 1) * NQT].rearrange(
                                    "p (h e) -> p h e", e=DH))

                # ---- phase 2b: word_q^T, Q^T, entity projections ----
                tr2 = ctxB.enter_context(tc.tile_pool(name="tr2", bufs=1))
                wqT = tr2.tile([P, HC, QWR], F32R)
                for o in range(QWR // P):
                    nat = natp.tile([P, H], F32, tag="nat", name="nat")
                    nc.sync.dma_start(nat, d["word_q"][o * P:(o + 1) * P, :])
                    for c in range(HC):
                        transpose_block(wqT[:, c, o * P:(o + 1) * P],
                                        nat[:, c * P:(c + 1) * P], P)

                for q in range(H // NQT):
                    wr = load_w("Wq", q)
                    for mloc in range(NQT // P):
                        m = q * (NQT // P) + mloc
                        ps = prps.tile([P, 512], F32, tag="pr", name="psq")
                        for k in range(HC):
                            nc.tensor.matmul(
                                ps, wr[:, k, mloc * P:(mloc + 1) * P],
                                wqT[:, k, :], start=(k == 0), stop=(k == HC - 1))
                        nc.vector.tensor_scalar(
                            out=QT[:, m, :QWR], in0=ps,
                            scalar1=pbias["bq"][:, m:m + 1], scalar2=0.125,
                            op0=OP.add, op1=OP.mult)

                # entity transposes
                ent_nat = natp.tile([P, H], F32, tag="nat", name="nat")
                nc.sync.dma_start(ent_nat, d["ent"])
                qp_nat = natp.tile([P, H], F32, tag="nat", name="nat")
                nc.sync.dma_start(qp_nat, d["qpos"])
                pe_nat = tr2.tile([P, H], F32)
                nc.vector.tensor_add(pe_nat, ent_nat, qp_nat)
                nc.vector.tensor_scalar_mul(pe_nat, pe_nat, 0.5)
                posentT = tr2.tile([P, HC, P], F32R)
                entT = tr2.tile([P, HC, P], F32R)
                for c in range(HC):
                    transpose_block(posentT[:, c, :],
                                    pe_nat[:, c * P:(c + 1) * P], P)
                    transpose_block(entT[:, c, :],
                                    ent_nat[:, c * P:(c + 1) * P], P)

                eq_nat = natp.tile([P, H], F32, tag="nat", name="nat")
                nc.sync.dma_start(eq_nat[:QER, :], d["ent_q"])
                qq_nat = natp.tile([P, H], F32, tag="nat", name="nat")
                nc.sync.dma_start(qq_nat[:QER, :], d["qpos_q"])
                peq_nat = tr2.tile([QER, H], F32)
                nc.vector.tensor_add(peq_nat, eq_nat[:QER, :], qq_nat[:QER, :])
                nc.vector.tensor_scalar_mul(peq_nat, peq_nat, 0.5)
                peqT = tr2.tile([P, HC, QER], F32R)
                for c in range(HC):
                    transpose_block(peqT[:, c, :],
                                    peq_nat[:, c * P:(c + 1) * P], QER)

                # entity V: V[:, 8, :, :DH] = ent @ Wev + bev
                bevb = bias_bcast(bvp, "bev", "bv")
                for q in range(H // NQT):
                    wr = load_w("Wev", q)
                    ps = prps.tile([P, 512], F32, tag="pr", name="psve")[:, :NQT]
                    for k in range(HC):
                        nc.tensor.matmul(ps, entT[:, k, :], wr[:, k, :],
                                         start=(k == 0), stop=(k == HC - 1))
                    nc.vector.tensor_add(
                        V[:, KC - 1, q * 4:(q + 1) * 4, :DH],
                        ps.rearrange("p (h e) -> p h e", e=DH),
                        bevb[:, q * NQT:(q + 1) * NQT].rearrange(
                            "p (h e) -> p h e", e=DH))

                # entity K: ke = pos_ent @ Wek (natural, then PE-transpose)
                ke_r = tr2.tile([P, H], F32R)
                for q in range(H // NQT):
                    wr = load_w("Wek", q)
                    ps = prps.tile([P, 512], F32, tag="pr", name="pske")[:, :NQT]
                    for k in range(HC):
                        nc.tensor.matmul(ps, posentT[:, k, :], wr[:, k, :],
                                         start=(k == 0), stop=(k == HC - 1))
                    nc.vector.tensor_copy(ke_r[:, q * NQT:(q + 1) * NQT], ps)
                for c in range(HC):
                    transpose_block(
                        KT[:, c, TW:T], ke_r[:, c * P:(c + 1) * P], P,
                        use_r=True, scale_bias=(pbias["bek"][:, c:c + 1], 1.0))

                # entity Q: qe = pos_ent_q @ Weq (natural 64 rows, transpose)
                qe_r = tr2.tile([QER, H], F32R)
                for q in range(H // NQT):
                    wr = load_w("Weq", q)
                    ps = prps.tile([P, 512], F32, tag="pr",
                                   name="psqe")[:QER, :NQT]
                    for k in range(HC):
                        nc.tensor.matmul(ps, peqT[:, k, :], wr[:, k, :],
                                         start=(k == 0), stop=(k == HC - 1))
                    nc.vector.tensor_copy(qe_r[:, q * NQT:(q + 1) * NQT], ps)
                for c in range(HC):
                    transpose_block(
                        QT[:, c, QWR:NQ], qe_r[:, c * P:(c + 1) * P], QER,
                        use_r=True,
                        scale_bias=(pbias["beq"][:, c:c + 1], 0.125))

            # ================= phase 3: attention =================
            with ExitStack() as ctxC:
                ptp = ctxC.enter_context(tc.tile_pool(name="pt", bufs=2))
                rp = ctxC.enter_context(tc.tile_pool(name="rp", bufs=2))
                sps = ctxC.enter_context(
                    tc.tile_pool(name="sps", bufs=4, space="PSUM"))
                cps = ctxC.enter_context(
                    tc.tile_pool(name="cps", bufs=4, space="PSUM"))

                for nh in range(NH):
                    po = DH * (nh % 2)
                    chk = nh // 2
                    pt = ptp.tile([P, KC, NQ], F32R, tag="pt", name="pt")
                    for kc in range(KC):
                        for hq in range(2):
                            sp = sps.tile([P, QH], F32, tag="sp", name="sp")
                            nc.tensor.matmul(
                                sp,
                                KT[po:po + DH, chk, kc * P:(kc + 1) * P],
                                QT[po:po + DH, chk, hq * QH:(hq + 1) * QH],
                                start=True, stop=True)
                            nc.scalar.activation(
                                pt[:, kc, hq * QH:(hq + 1) * QH], sp, AF.Exp,
                                bias=maskT[:, kc:kc + 1], scale=1.0)
                    cp = [cps.tile([P, QH], F32, tag="cp", name=f"cp{i}")
                          for i in range(2)]
                    for kc in range(KC):
                        for hq in range(2):
                            nc.tensor.matmul(
                                cp[hq][:DH + 1, :], V[:, kc, nh, :],
                                pt[:, kc, hq * QH:(hq + 1) * QH],
                                start=(kc == 0), stop=(kc == KC - 1))
                    rec = rp.tile([1, NQ], F32, tag="rec", name="rec")
                    for hq in range(2):
                        nc.vector.reciprocal(
                            rec[:, hq * QH:(hq + 1) * QH], cp[hq][DH:DH + 1, :])
                    rec_r = rp.tile([1, NQ], F32R, tag="recr", name="rec_r")
                    nc.vector.tensor_copy(rec_r, rec)
                    for hq in range(2):
                        bc = sps.tile([P, QH], F32, tag="sp", name="bc")
                        nc.tensor.matmul(
                            bc[:DH, :], ones_r[:1, :DH],
                            rec_r[:1, hq * QH:(hq + 1) * QH],
                            start=True, stop=True)
                        bcs = rp.tile([DH, QH], F32, tag="bcs", name="bcs")
                        nc.vector.tensor_copy(bcs, bc[:DH, :])
                        nc.vector.tensor_mul(
                            ctxT[po:po + DH, chk, hq * QH:(hq + 1) * QH],
                            cp[hq][:DH, :], bcs)

        # ================= phase 4: output projections + LN =================
        with ExitStack() as ctxD:
            wt32b = ctxD.enter_context(tc.tile_pool(name="wt32b", bufs=1))
            wtrb = ctxD.enter_context(tc.tile_pool(name="wtrb", bufs=1))
            yp = ctxD.enter_context(tc.tile_pool(name="yp", bufs=3))
            resp = ctxD.enter_context(tc.tile_pool(name="resp", bufs=2))
            gbp = ctxD.enter_context(tc.tile_pool(name="gbp", bufs=1))
            stat = ctxD.enter_context(tc.tile_pool(name="stat", bufs=4))
            opps = ctxD.enter_context(
                tc.tile_pool(name="opps", bufs=4, space="PSUM"))

            def layernorm(y, rows, g_sb, b_sb):
                st = stat.tile([P, 2, 6], F32, tag="st", name="st")
                for sgi in range(2):
                    nc.vector.bn_stats(st[:rows, sgi, :],
                                       y[:rows, sgi * 512:(sgi + 1) * 512])
                mv = stat.tile([P, 2], F32, tag="mv", name="mv")
                nc.vector.bn_aggr(mv[:rows], st[:rows])
                sd = stat.tile([P, 1], F32, tag="sd", name="sd")
                nc.scalar.activation(sd[:rows], mv[:rows, 1:2], AF.Sqrt,
                                     bias=eps_t[:rows], scale=1.0)
                rs = stat.tile([P, 1], F32, tag="rs", name="rs")
                nc.vector.reciprocal(rs[:rows], sd[:rows])
                nc.vector.tensor_scalar(
                    out=y[:rows], in0=y[:rows], scalar1=mv[:rows, 0:1],
                    scalar2=rs[:rows], op0=OP.subtract, op1=OP.mult)
                nc.vector.tensor_mul(y[:rows], y[:rows], g_sb[:rows])
                nc.vector.tensor_add(y[:rows], y[:rows], b_sb[:rows])

            def out_block(wname, bname, gname, bnname, resname, outname, rows,
                          nt, ctx_lo):
                w32 = wt32b.tile([P, HC, H], F32, tag="w", name="w32")
                nc.sync.dma_start(w32, d[wname].rearrange("(k p) f -> p k f",
                                                          p=P))
                w_r = wtrb.tile([P, HC, H], F32R, tag="wr", name="w_r")
                for kk in range(0, HC, 2):
                    nc.gpsimd.tensor_copy(w_r[:, kk:kk + 2, :],
                                          w32[:, kk:kk + 2, :])
                bob = bias_bcast(gbp, bname, "bias")
                g_sb = bias_bcast(gbp, gname, "g")
                b_sb = bias_bcast(gbp, bnname, "b")
                for t in range(nt):
                    res = resp.tile([P, H], F32, tag="res", name="res")
                    nc.sync.dma_start(res[:rows, :],
                                      d[resname][t * P:t * P + rows, :])
                    y = yp.tile([P, H], F32, tag="y", name="y")
                    for n in range(H // 512):
                        ps = opps.tile([P, 512], F32, tag="op", name="op")
                        for k in range(HC):
                            nc.tensor.matmul(
                                ps[:rows, :],
                                ctxT[:, k, ctx_lo + t * P:ctx_lo + t * P + rows],
                                w_r[:, k, n * 512:(n + 1) * 512],
                                start=(k == 0), stop=(k == HC - 1))
                        nc.vector.tensor_add(
                            y[:rows, n * 512:(n + 1) * 512], ps[:rows, :],
                            res[:rows, n * 512:(n + 1) * 512])
                        nc.vector.tensor_add(
                            y[:rows, n * 512:(n + 1) * 512],
                            y[:rows, n * 512:(n + 1) * 512],
                            bob[:rows, n * 512:(n + 1) * 512])
                    layernorm(y, rows, g_sb, b_sb)
                    nc.sync.dma_start(d[outname][t * P:t * P + rows, :],
                                      y[:rows, :])

            out_block("Wo", "bo", "ln_g", "ln_b", "word_q", "word_out",
                      P, QWR // P, 0)
            out_block("Weo", "beo", "eln_g", "eln_b", "ent_q", "ent_out",
                      QER, 1, QWR)


def kernel(**inputs):
    global LAST_RESULTS
    if "nc" not in _STATE:
        _STATE["nc"] = _build()
    nc = _STATE["nc"]

    word = np.asarray(inputs["word_hidden_states"], np.float32)
    ent = np.asarray(inputs["entity_hidden_states"], np.float32)
    qpos = np.asarray(inputs["query_pos"], np.float32)
    mask = np.asarray(inputs["attention_mask"], np.float32)
    B = word.shape[0]
    c = np.ascontiguousarray

    shared = {}
    for w in ["Wq", "Wk", "Wv", "Weq", "Wek", "Wev", "Wo", "Weo"]:
        shared[w] = c(np.asarray(inputs[w], np.float32))
    for b in ["bq", "bk", "bv", "beq", "bek", "bev", "bo", "beo"]:
        shared[b] = c(np.asarray(inputs[b], np.float32))
    for g in ["ln_g", "ln_b", "eln_g", "eln_b"]:
        shared[g] = c(np.asarray(inputs[g], np.float32))

    in_maps = []
    for core in range(8):
        b, h = core // 2, core % 2
        m = dict(shared)
        m["word"] = c(word[b])
        m["ent"] = c(ent[b])
        m["qpos"] = c(qpos[b])
        m["word_q"] = c(word[b, h * QWR:(h + 1) * QWR])
        m["ent_q"] = c(ent[b, h * QER:(h + 1) * QER])
        m["qpos_q"] = c(qpos[b, h * QER:(h + 1) * QER])
        m["mask"] = c(mask[b, 0, 0])
        in_maps.append(m)

    res = run_bass_kernel_spmd(nc, in_maps, core_ids=list(range(8)))
    LAST_RESULTS = res

    word_out = np.zeros((B, TW, H), np.float32)
    ent_out = np.zeros((B, TE, H), np.float32)
    for core in range(8):
        b, h = core // 2, core % 2
        word_out[b, h * QWR:(h + 1) * QWR] = res.results[core]["word_out"]
        ent_out[b, h * QER:(h + 1) * QER] = res.results[core]["ent_out"]
    return word_out, ent_out


# revision 10
# speedup vs baseline: 1.0992x; 1.0992x over previous
"""Entity-aware BERT attention (LUKE-style) Trainium2 Bass kernel.

Sharding: 8 cores = 4 batches x 2 query-halves. Each core computes the full
K/V for its batch (duplicated across the pair of cores sharing a batch, which
avoids any cross-core collective), then attention + output projection +
LayerNorm for its own 512 word rows and 64 entity rows.

All matmuls run in bf16 (full PE rate + fast weight load, which keeps the
PE-array duty cycle high enough that the HAM clock gate stays at 2.4GHz;
fp32/fp32r weight loads are 4x slower and left the PE throttled). Activations
are kept transposed (hidden on partitions) so projections and attention feed
the tensor engine directly; scores are computed transposed (S^T[kv, q]) so
the softmax exp output is consumed as the matmul moving operand with no
transpose. The softmax denominator comes from an appended ones-column in V,
and the per-query division uses a K=1 matmul to broadcast 1/denom across
partitions. Free-dim biases are applied via partition-broadcast DMA tiles.
"""
import os
import sys

sys.path.insert(0, "/opt/trn_rl_repo")

import numpy as np  # noqa: E402

import concourse.bass as bass  # noqa: E402,F401
import concourse.tile as tile  # noqa: E402
from concourse import bacc, mybir  # noqa: E402
from concourse.bass_utils import run_bass_kernel_spmd  # noqa: E402
from concourse.masks import make_identity  # noqa: E402
from contextlib import ExitStack  # noqa: E402

F32 = mybir.dt.float32
BF16 = mybir.dt.bfloat16
AF = mybir.ActivationFunctionType
OP = mybir.AluOpType

P = 128
H = 1024
NH = 16
DH = 64
TW = 1024        # word tokens
TE = 128         # entity tokens
T = TW + TE      # 1152 keys
QWR = 512        # word query rows per core
QER = 64         # entity query rows per core
NQ = QWR + QER   # 576 queries per core
HC = H // P      # 8 hidden chunks
KC = T // P      # 9 kv chunks
QH = NQ // 2     # 288, matmul free-dim half (>=256 keeps fp32r at full rate)
NQT = 256        # weight streaming quarter width
EPS = 1e-12

_STATE = {}
LAST_RESULTS = None


def _build():
    nc = bacc.Bacc("TRN2", target_bir_lowering=False, debug=False, num_devices=8)

    d = {}
    def din(name, shape):
        d[name] = nc.dram_tensor(name, shape, F32, kind="ExternalInput").ap()
    def dout(name, shape):
        d[name] = nc.dram_tensor(name, shape, F32, kind="ExternalOutput").ap()

    din("word", [TW, H]); din("ent", [TE, H]); din("qpos", [TE, H])
    din("word_q", [QWR, H]); din("ent_q", [QER, H]); din("qpos_q", [QER, H])
    din("mask", [T])
    for w in ["Wq", "Wk", "Wv", "Weq", "Wek", "Wev", "Wo", "Weo"]:
        din(w, [H, H])
    for b in ["bq", "bk", "bv", "beq", "bek", "bev", "bo", "beo"]:
        din(b, [H])
    for g in ["ln_g", "ln_b", "eln_g", "eln_b"]:
        din(g, [H])
    dout("word_out", [QWR, H])
    dout("ent_out", [QER, H])

    with tile.TileContext(nc) as tc:
        _emit(nc, tc, d)
    nc.compile()
    return nc


def _emit(nc, tc, d):
    with ExitStack() as ctx0:
        const = ctx0.enter_context(tc.tile_pool(name="const", bufs=1))
        ctxp = ctx0.enter_context(tc.tile_pool(name="ctxp", bufs=1))

        # ---- constants ----
        ident = const.tile([P, P], F32)
        make_identity(nc, ident)
        ident_r = const.tile([P, P], BF16)
        nc.gpsimd.tensor_copy(ident_r, ident)
        ones_f = const.tile([1, P], F32)
        nc.vector.memset(ones_f, 1.0)
        ones_r = const.tile([1, P], BF16)
        nc.vector.tensor_copy(ones_r, ones_f)
        eps_t = const.tile([P, 1], F32)
        nc.vector.memset(eps_t, EPS)

        maskT = const.tile([P, KC], F32)
        nc.sync.dma_start(maskT, d["mask"].rearrange("(c p) -> p c", p=P))

        # per-partition (feature-on-partition) bias views for transposed outs
        pbias = {}
        for b in ["bk", "bq", "bek", "beq"]:
            t = const.tile([P, HC], F32, name=f"pb_{b}")
            nc.sync.dma_start(t, d[b].rearrange("(m p) -> p m", p=P))
            pbias[b] = t

        def bias_bcast(pool, name, tag):
            # [P, H] partition-broadcast copy of a [H] bias row
            t = pool.tile([P, H], F32, tag=tag, name=f"bb_{name}")
            nc.sync.dma_start(t, d[name][None, :].to_broadcast((P, H)))
            return t

        ctxT = ctxp.tile([P, HC, NQ], BF16)     # attention output^T [feat, q]

        with ExitStack() as ctxKVQ:
            kvq = ctxKVQ.enter_context(tc.tile_pool(name="kvq", bufs=1))
            KT = kvq.tile([P, HC, T], BF16)          # K^T  [feat, kv]
            V = kvq.tile([P, KC, NH, DH + 1], BF16)  # V (+ones col)
            QT = kvq.tile([P, HC, NQ], BF16)         # Q^T / 8  [feat, q]
            vone_f = const.tile([P, KC, NH, 1], F32)
            nc.vector.memset(vone_f, 1.0)
            nc.vector.tensor_copy(V[:, :, :, DH:DH + 1], vone_f)

            # ============ phase 1+2: transposes & projections ============
            with ExitStack() as ctxB:
                natp = ctxB.enter_context(tc.tile_pool(name="nat", bufs=2))
                wt32 = ctxB.enter_context(tc.tile_pool(name="wt32", bufs=2))
                wtr = ctxB.enter_context(tc.tile_pool(name="wtr", bufs=2))
                bvp = ctxB.enter_context(tc.tile_pool(name="bvp", bufs=2))
                tpps = ctxB.enter_context(
                    tc.tile_pool(name="tpps", bufs=2, space="PSUM"))
                prps = ctxB.enter_context(
                    tc.tile_pool(name="prps", bufs=4, space="PSUM"))

                def transpose_block(dst_ap, src_ap, rows, use_r=False,
                                    scale_bias=None):
                    ps = tpps.tile([P, P], BF16 if use_r else F32, tag="tp",
                                   name="tps")
                    idt = ident_r if use_r else ident
                    nc.tensor.transpose(ps[:, :rows], src_ap, idt[:rows, :rows])
                    if scale_bias is None:
                        nc.vector.tensor_copy(dst_ap, ps[:, :rows])
                    else:
                        b_ap, sc = scale_bias
                        nc.vector.tensor_scalar(
                            out=dst_ap, in0=ps[:, :rows], scalar1=b_ap,
                            scalar2=sc, op0=OP.add, op1=OP.mult)

                def load_w(name, q):
                    wt = wt32.tile([P, HC, NQT], F32, tag="wt", name="wt")
                    nc.sync.dma_start(
                        wt, d[name].rearrange("(k p) f -> p k f", p=P)
                        [:, :, q * NQT:(q + 1) * NQT])
                    wr = wtr.tile([P, HC, NQT], BF16, tag="wr", name="wr")
                    nc.gpsimd.tensor_copy(wr, wt)
                    return wr

                # ---- phase 2a: word^T, K^T, V (wordT freed after) ----
                with ExitStack() as ctxA:
                    wordT_pool = ctxA.enter_context(
                        tc.tile_pool(name="wordT", bufs=1))
                    wordT = wordT_pool.tile([P, HC, TW], BF16)
                    for o in range(HC):
                        nat = natp.tile([P, H], F32, tag="nat", name="nat")
                        nc.sync.dma_start(nat, d["word"][o * P:(o + 1) * P, :])
                        for c in range(HC):
                            transpose_block(wordT[:, c, o * P:(o + 1) * P],
                                            nat[:, c * P:(c + 1) * P], P)

                    # K^T = (word @ Wk)^T + bk -> KT[:, m, 0:TW]
                    for q in range(H // NQT):
                        wr = load_w("Wk", q)
                        for mloc in range(NQT // P):
                            m = q * (NQT // P) + mloc
                            for n in range(TW // 512):
                                ps = prps.tile([P, 512], F32, tag="pr",
                                               name="psk")
                                for k in range(HC):
                                    nc.tensor.matmul(
                                        ps, wr[:, k, mloc * P:(mloc + 1) * P],
                                        wordT[:, k, n * 512:(n + 1) * 512],
                                        start=(k == 0), stop=(k == HC - 1))
                                nc.vector.tensor_scalar_add(
                                    out=KT[:, m, n * 512:(n + 1) * 512],
                                    in0=ps, scalar1=pbias["bk"][:, m:m + 1])

                    # V = word @ Wv + bv (natural) -> V[:, t, :, 0:DH]
                    bvb = bias_bcast(bvp, "bv", "bv")
                    for q in range(H // NQT):
                        wr = load_w("Wv", q)
                        for t in range(TW // P):
                            ps = prps.tile([P, 512], F32, tag="pr",
                                           name="psv")[:, :NQT]
                            for k in range(HC):
                                nc.tensor.matmul(
                                    ps, wordT[:, k, t * P:(t + 1) * P],
                                    wr[:, k, :],
                                    start=(k == 0), stop=(k == HC - 1))
                            nc.vector.tensor_add(
                                V[:, t, q * 4:(q + 1) * 4, :DH],
                                ps.rearrange("p (h e) -> p h e", e=DH),
                                bvb[:, q * NQT:(q + 1) * NQT].rearrange(
                                    "p (h e) -> p h e", e=DH))

                # ---- phase 2b: word_q^T, Q^T, entity projections ----
                tr2 = ctxB.enter_context(tc.tile_pool(name="tr2", bufs=1))
                wqT = tr2.tile([P, HC, QWR], BF16)
                for o in range(QWR // P):
                    nat = natp.tile([P, H], F32, tag="nat", name="nat")
                    nc.sync.dma_start(nat, d["word_q"][o * P:(o + 1) * P, :])
                    for c in range(HC):
                        transpose_block(wqT[:, c, o * P:(o + 1) * P],
                                        nat[:, c * P:(c + 1) * P], P)

                for q in range(H // NQT):
                    wr = load_w("Wq", q)
                    for mloc in range(NQT // P):
                        m = q * (NQT // P) + mloc
                        ps = prps.tile([P, 512], F32, tag="pr", name="psq")
                        for k in range(HC):
                            nc.tensor.matmul(
                                ps, wr[:, k, mloc * P:(mloc + 1) * P],
                                wqT[:, k, :], start=(k == 0), stop=(k == HC - 1))
                        nc.vector.tensor_scalar(
                            out=QT[:, m, :QWR], in0=ps,
                            scalar1=pbias["bq"][:, m:m + 1], scalar2=0.125,
                            op0=OP.add, op1=OP.mult)

                # entity transposes
                ent_nat = natp.tile([P, H], F32, tag="nat", name="nat")
                nc.sync.dma_start(ent_nat, d["ent"])
                qp_nat = natp.tile([P, H], F32, tag="nat", name="nat")
                nc.sync.dma_start(qp_nat, d["qpos"])
                pe_nat = tr2.tile([P, H], F32)
                nc.vector.tensor_add(pe_nat, ent_nat, qp_nat)
                nc.vector.tensor_scalar_mul(pe_nat, pe_nat, 0.5)
                posentT = tr2.tile([P, HC, P], BF16)
                entT = tr2.tile([P, HC, P], BF16)
                for c in range(HC):
                    transpose_block(posentT[:, c, :],
                                    pe_nat[:, c * P:(c + 1) * P], P)
                    transpose_block(entT[:, c, :],
                                    ent_nat[:, c * P:(c + 1) * P], P)

                eq_nat = natp.tile([P, H], F32, tag="nat", name="nat")
                nc.sync.dma_start(eq_nat[:QER, :], d["ent_q"])
                qq_nat = natp.tile([P, H], F32, tag="nat", name="nat")
                nc.sync.dma_start(qq_nat[:QER, :], d["qpos_q"])
                peq_nat = tr2.tile([QER, H], F32)
                nc.vector.tensor_add(peq_nat, eq_nat[:QER, :], qq_nat[:QER, :])
                nc.vector.tensor_scalar_mul(peq_nat, peq_nat, 0.5)
                peqT = tr2.tile([P, HC, QER], BF16)
                for c in range(HC):
                    transpose_block(peqT[:, c, :],
                                    peq_nat[:, c * P:(c + 1) * P], QER)

                # entity V: V[:, 8, :, :DH] = ent @ Wev + bev
                bevb = bias_bcast(bvp, "bev", "bv")
                for q in range(H // NQT):
                    wr = load_w("Wev", q)
                    ps = prps.tile([P, 512], F32, tag="pr", name="psve")[:, :NQT]
                    for k in range(HC):
                        nc.tensor.matmul(ps, entT[:, k, :], wr[:, k, :],
                                         start=(k == 0), stop=(k == HC - 1))
                    nc.vector.tensor_add(
                        V[:, KC - 1, q * 4:(q + 1) * 4, :DH],
                        ps.rearrange("p (h e) -> p h e", e=DH),
                        bevb[:, q * NQT:(q + 1) * NQT].rearrange(
                            "p (h e) -> p h e", e=DH))

                # entity K: ke = pos_ent @ Wek (natural, then PE-transpose)
                ke_r = tr2.tile([P, H], BF16)
                for q in range(H // NQT):
                    wr = load_w("Wek", q)
                    ps = prps.tile([P, 512], F32, tag="pr", name="pske")[:, :NQT]
                    for k in range(HC):
                        nc.tensor.matmul(ps, posentT[:, k, :], wr[:, k, :],
                                         start=(k == 0), stop=(k == HC - 1))
                    nc.vector.tensor_copy(ke_r[:, q * NQT:(q + 1) * NQT], ps)
                for c in range(HC):
                    transpose_block(
                        KT[:, c, TW:T], ke_r[:, c * P:(c + 1) * P], P,
                        use_r=True, scale_bias=(pbias["bek"][:, c:c + 1], 1.0))

                # entity Q: qe = pos_ent_q @ Weq (natural 64 rows, transpose)
                qe_r = tr2.tile([QER, H], BF16)
                for q in range(H // NQT):
                    wr = load_w("Weq", q)
                    ps = prps.tile([P, 512], F32, tag="pr",
                                   name="psqe")[:QER, :NQT]
                    for k in range(HC):
                        nc.tensor.matmul(ps, peqT[:, k, :], wr[:, k, :],
                                         start=(k == 0), stop=(k == HC - 1))
                    nc.vector.tensor_copy(qe_r[:, q * NQT:(q + 1) * NQT], ps)
                for c in range(HC):
                    transpose_block(
                        QT[:, c, QWR:NQ], qe_r[:, c * P:(c + 1) * P], QER,
                        use_r=True,
                        scale_bias=(pbias["beq"][:, c:c + 1], 0.125))

            # ================= phase 3: attention =================
            with ExitStack() as ctxC:
                ptp = ctxC.enter_context(tc.tile_pool(name="pt", bufs=2))
                rp = ctxC.enter_context(tc.tile_pool(name="rp", bufs=2))
                sps = ctxC.enter_context(
                    tc.tile_pool(name="sps", bufs=4, space="PSUM"))
                cps = ctxC.enter_context(
                    tc.tile_pool(name="cps", bufs=4, space="PSUM"))

                for nh in range(NH):
                    po = DH * (nh % 2)
                    chk = nh // 2
                    pt = ptp.tile([P, KC, NQ], BF16, tag="pt", name="pt")
                    for kc in range(KC):
                        for hq in range(2):
                            sp = sps.tile([P, QH], F32, tag="sp", name="sp")
                            nc.tensor.matmul(
                                sp,
                                KT[po:po + DH, chk, kc * P:(kc + 1) * P],
                                QT[po:po + DH, chk, hq * QH:(hq + 1) * QH],
                                start=True, stop=True)
                            nc.scalar.activation(
                                pt[:, kc, hq * QH:(hq + 1) * QH], sp, AF.Exp,
                                bias=maskT[:, kc:kc + 1], scale=1.0)
                    cp = [cps.tile([P, QH], F32, tag="cp", name=f"cp{i}")
                          for i in range(2)]
                    for kc in range(KC):
                        for hq in range(2):
                            nc.tensor.matmul(
                                cp[hq][:DH + 1, :], V[:, kc, nh, :],
                                pt[:, kc, hq * QH:(hq + 1) * QH],
                                start=(kc == 0), stop=(kc == KC - 1))
                    rec = rp.tile([1, NQ], F32, tag="rec", name="rec")
                    for hq in range(2):
                        nc.vector.reciprocal(
                            rec[:, hq * QH:(hq + 1) * QH], cp[hq][DH:DH + 1, :])
                    rec_r = rp.tile([1, NQ], BF16, tag="recr", name="rec_r")
                    nc.vector.tensor_copy(rec_r, rec)
                    for hq in range(2):
                        bc = sps.tile([P, QH], F32, tag="sp", name="bc")
                        nc.tensor.matmul(
                            bc[:DH, :], ones_r[:1, :DH],
                            rec_r[:1, hq * QH:(hq + 1) * QH],
                            start=True, stop=True)
                        bcs = rp.tile([DH, QH], F32, tag="bcs", name="bcs")
                        nc.vector.tensor_copy(bcs, bc[:DH, :])
                        nc.vector.tensor_mul(
                            ctxT[po:po + DH, chk, hq * QH:(hq + 1) * QH],
                            cp[hq][:DH, :], bcs)

        # ================= phase 4: output projections + LN =================
        with ExitStack() as ctxD:
            wt32b = ctxD.enter_context(tc.tile_pool(name="wt32b", bufs=1))
            wtrb = ctxD.enter_context(tc.tile_pool(name="wtrb", bufs=1))
            yp = ctxD.enter_context(tc.tile_pool(name="yp", bufs=3))
            resp = ctxD.enter_context(tc.tile_pool(name="resp", bufs=2))
            gbp = ctxD.enter_context(tc.tile_pool(name="gbp", bufs=1))
            stat = ctxD.enter_context(tc.tile_pool(name="stat", bufs=4))
            opps = ctxD.enter_context(
                tc.tile_pool(name="opps", bufs=4, space="PSUM"))

            def layernorm(y, rows, g_sb, b_sb):
                st = stat.tile([P, 2, 6], F32, tag="st", name="st")
                for sgi in range(2):
                    nc.vector.bn_stats(st[:rows, sgi, :],
                                       y[:rows, sgi * 512:(sgi + 1) * 512])
                mv = stat.tile([P, 2], F32, tag="mv", name="mv")
                nc.vector.bn_aggr(mv[:rows], st[:rows])
                sd = stat.tile([P, 1], F32, tag="sd", name="sd")
                nc.scalar.activation(sd[:rows], mv[:rows, 1:2], AF.Sqrt,
                                     bias=eps_t[:rows], scale=1.0)
                rs = stat.tile([P, 1], F32, tag="rs", name="rs")
                nc.vector.reciprocal(rs[:rows], sd[:rows])
                nc.vector.tensor_scalar(
                    out=y[:rows], in0=y[:rows], scalar1=mv[:rows, 0:1],
                    scalar2=rs[:rows], op0=OP.subtract, op1=OP.mult)
                nc.vector.tensor_mul(y[:rows], y[:rows], g_sb[:rows])
                nc.vector.tensor_add(y[:rows], y[:rows], b_sb[:rows])

            def out_block(wname, bname, gname, bnname, resname, outname, rows,
                          nt, ctx_lo):
                w32 = wt32b.tile([P, HC, H], F32, tag="w", name="w32")
                nc.sync.dma_start(w32, d[wname].rearrange("(k p) f -> p k f",
                                                          p=P))
                w_r = wtrb.tile([P, HC, H], BF16, tag="wr", name="w_r")
                for kk in range(0, HC, 2):
                    nc.gpsimd.tensor_copy(w_r[:, kk:kk + 2, :],
                                          w32[:, kk:kk + 2, :])
                bob = bias_bcast(gbp, bname, "bias")
                g_sb = bias_bcast(gbp, gname, "g")
                b_sb = bias_bcast(gbp, bnname, "b")
                for t in range(nt):
                    res = resp.tile([P, H], F32, tag="res", name="res")
                    nc.sync.dma_start(res[:rows, :],
                                      d[resname][t * P:t * P + rows, :])
                    y = yp.tile([P, H], F32, tag="y", name="y")
                    for n in range(H // 512):
                        ps = opps.tile([P, 512], F32, tag="op", name="op")
                        for k in range(HC):
                            nc.tensor.matmul(
                                ps[:rows, :],
                                ctxT[:, k, ctx_lo + t * P:ctx_lo + t * P + rows],
                                w_r[:, k, n * 512:(n + 1) * 512],
                                start=(k == 0), stop=(k == HC - 1))
                        nc.vector.tensor_add(
                            y[:rows, n * 512:(n + 1) * 512], ps[:rows, :],
                            res[:rows, n * 512:(n + 1) * 512])
                        nc.vector.tensor_add(
                            y[:rows, n * 512:(n + 1) * 512],
                            y[:rows, n * 512:(n + 1) * 512],
                            bob[:rows, n * 512:(n + 1) * 512])
                    layernorm(y, rows, g_sb, b_sb)
                    nc.sync.dma_start(d[outname][t * P:t * P + rows, :],
                                      y[:rows, :])

            out_block("Wo", "bo", "ln_g", "ln_b", "word_q", "word_out",
                      P, QWR // P, 0)
            out_block("Weo", "beo", "eln_g", "eln_b", "ent_q", "ent_out",
                      QER, 1, QWR)


def kernel(**inputs):
    global LAST_RESULTS
    if "nc" not in _STATE:
        _STATE["nc"] = _build()
    nc = _STATE["nc"]

    word = np.asarray(inputs["word_hidden_states"], np.float32)
    ent = np.asarray(inputs["entity_hidden_states"], np.float32)
    qpos = np.asarray(inputs["query_pos"], np.float32)
    mask = np.asarray(inputs["attention_mask"], np.float32)
    B = word.shape[0]
    c = np.ascontiguousarray

    shared = {}
    for w in ["Wq", "Wk", "Wv", "Weq", "Wek", "Wev", "Wo", "Weo"]:
        shared[w] = c(np.asarray(inputs[w], np.float32))
    for b in ["bq", "bk", "bv", "beq", "bek", "bev", "bo", "beo"]:
        shared[b] = c(np.asarray(inputs[b], np.float32))
    for g in ["ln_g", "ln_b", "eln_g", "eln_b"]:
        shared[g] = c(np.asarray(inputs[g], np.float32))

    in_maps = []
    for core in range(8):
        b, h = core // 2, core % 2
        m = dict(shared)
        m["word"] = c(word[b])
        m["ent"] = c(ent[b])
        m["qpos"] = c(qpos[b])
        m["word_q"] = c(word[b, h * QWR:(h + 1) * QWR])
        m["ent_q"] = c(ent[b, h * QER:(h + 1) * QER])
        m["qpos_q"] = c(qpos[b, h * QER:(h + 1) * QER])
        m["mask"] = c(mask[b, 0, 0])
        in_maps.append(m)

    res = run_bass_kernel_spmd(nc, in_maps, core_ids=list(range(8)))
    LAST_RESULTS = res

    word_out = np.zeros((B, TW, H), np.float32)
    ent_out = np.zeros((B, TE, H), np.float32)
    for core in range(8):
        b, h = core // 2, core % 2
        word_out[b, h * QWR:(h + 1) * QWR] = res.results[core]["word_out"]
        ent_out[b, h * QER:(h + 1) * QER] = res.results[core]["ent_out"]
    return word_out, ent_out
